# revision 1
# baseline (speedup 1.0000x reference)
"""Multi-head self-attention TRN2 Bass kernel (8-core SPMD).

Problem: z [4, 2048, 1024], w_q/w_k/w_v/w_o [1024, 1024] (torch Linear
convention: q = z @ w_q.T), b_o [1024]. 16 heads x 64 dims, softmax scale
1/sqrt(64).

Sharding: 8 cores = (4 batches) x (2 query-halves). Each core computes full
K/V for its batch (projection duplicated 2x across the query-half pair) and
attention + output projection for its 1024 queries. No collectives; host
concatenates per-core outputs.

Device-side layout: everything is computed transposed (contraction dim on
partitions). Host feeds z[b].T with the core's query tokens permuted to the
END of the token axis, so the query slice is a fixed (SPMD-identical) column
range. Softmax is unnormalized flash-style: exp(S) streams straight into the
AV matmul whose stationary operand carries an extra all-ones column that
accumulates the denominators; normalization happens at PSUM eviction.
V (augmented) and O.T round-trip through DRAM scratch to fit SBUF.
"""

import os
import sys

import numpy as np

for _p in ("/opt/trn_rl_repo", "/root/.axon_site/_ro/trn_rl_repo"):
    if os.path.isdir(_p) and _p not in sys.path:
        sys.path.insert(0, _p)

import concourse.bacc as bacc
import concourse.mybir as mybir
import concourse.tile as tile
from concourse import bass_utils

F32 = mybir.dt.float32
F32R = mybir.dt.float32r
P = 128


def full_cfg():
    return dict(EMB=1024, N=2048, NQ=1024, H=16, DH=64)


def small_cfg():
    return dict(EMB=256, N=256, NQ=128, H=4, DH=64)


def build_program(nc, cfg):
    EMB, N, NQ, H, DH = cfg["EMB"], cfg["N"], cfg["NQ"], cfg["H"], cfg["DH"]
    EC = EMB // P            # emb contraction chunks
    TC = N // P              # key-token chunks
    PAIRS = H // 2           # head pairs (128 dims each)
    EMBH = EMB // 2          # V computed in two dout halves
    H2 = EMBH // DH          # heads per V half
    QB = min(512, NQ)        # query block (matmul moving size)
    NQB = NQ // QB
    PW = min(512, EMBH)      # V psum width
    SCALE = 1.0 / np.sqrt(DH)
    DA = DH + 1              # V head dims + ones column

    zt_d = nc.dram_tensor("zt", [EMB, N], F32R, kind="ExternalInput").ap()
    wq_d = nc.dram_tensor("wq", [EMB, EMB], F32R, kind="ExternalInput").ap()
    wk_d = nc.dram_tensor("wk", [EMB, EMB], F32R, kind="ExternalInput").ap()
    wv_d = nc.dram_tensor("wv", [EMB, EMB], F32R, kind="ExternalInput").ap()
    wo_d = nc.dram_tensor("wo", [EMB, EMB], F32R, kind="ExternalInput").ap()
    bo_d = nc.dram_tensor("bo", [EMB], F32, kind="ExternalInput").ap()
    yt_d = nc.dram_tensor("yt", [EMB, NQ], F32, kind="ExternalOutput").ap()

    def rearr(ap):  # [EMB, X] dram -> [P, EC, X] partition view
        return ap.rearrange("(eo p) x -> p eo x", p=P)

    # ST head slices sit at a fixed 512 stride so the two row-packed matmuls
    # always drain into different PSUM banks.
    BIGW = max(PW, min(1024, NQ), min(1024, N), 512 + QB)

    with tile.TileContext(nc) as tc:
        with (
            tc.tile_pool(name="dram", bufs=1, space="DRAM") as dram,
            tc.tile_pool(name="const", bufs=1) as const,
            tc.tile_pool(name="wvp", bufs=1) as wvp,
            tc.tile_pool(name="stg", bufs=2) as stg,
            tc.tile_pool(name="attn", bufs=2) as attn,
            tc.tile_pool(name="expp", bufs=3) as expp,
            tc.tile_pool(name="tmp", bufs=2) as tmp,
            tc.tile_pool(name="tmp1", bufs=1) as tmp1,
        ):
            vaug_dram = dram.tile([TC, P, H, DA], F32R)   # V + ones column
            ot_dram = dram.tile([EMB, NQ], F32R)          # normalized O.T

            zt_t = []
            for _ec in range(EC):
                zte = const.tile([P, N], F32R, tag=f"zt{_ec}", name=f"zt{_ec}")
                nc.sync.dma_start(zte[:], rearr(zt_d)[:, _ec, :])
                zt_t.append(zte)
            bo_sb = const.tile([P, EC], F32)
            nc.sync.dma_start(bo_sb[:], bo_d.rearrange("(mo p) -> p mo", p=P))
            ones_row = const.tile([1, DH], F32R)
            nc.any.memset(ones_row[:].bitcast(F32), 1.0)
            REP = cfg.get("REP", 1)

            def emit_v_half(hf):
                wv_t = []
                for _ec in range(EC):
                    wve = wvp.tile([P, EMBH], F32R, tag=f"wv{_ec}", name=f"wv{_ec}")
                    nc.sync.dma_start(wve[:], rearr(wv_d)[:, _ec, hf * EMBH : (hf + 1) * EMBH])
                    wv_t.append(wve)
                for tci in range(TC):
                    for pwb in range(EMBH // PW):
                        ps = big_ps.tile([P, BIGW], F32, tag="big")
                        for ec in range(EC):
                            nc.tensor.matmul(
                                ps[:, :PW],
                                lhsT=zt_t[ec][:, tci * P : (tci + 1) * P],
                                rhs=wv_t[ec][:, pwb * PW : (pwb + 1) * PW],
                                start=(ec == 0),
                                stop=(ec == EC - 1),
                            )
                        nh = PW // DH
                        h0 = hf * H2 + pwb * nh
                        vs = stg.tile([P, nh, DA], F32R, tag="vs")
                        nc.vector.tensor_copy(
                            vs[:, :, 0:DH],
                            ps[:, :PW].rearrange("p (h d) -> p h d", d=DH),
                        )
                        nc.any.memset(vs[:, :, DH:DA].bitcast(F32), 1.0)
                        nc.sync.dma_start(vaug_dram[tci, :, h0 : h0 + nh, :], vs[:])

            def emit_kq(pair):
                wk_t = attn.tile([P, EC, P], F32R, tag="wk")
                nc.sync.dma_start(wk_t[:], rearr(wk_d)[:, :, pair * P : (pair + 1) * P])
                kt = attn.tile([P, N], F32R, tag="kt")
                RW = min(1024, N)
                for rnd in range(N // RW):
                    ps = big_ps.tile([P, BIGW], F32, tag="big")
                    for ec in range(EC):
                        for nb in range(RW // min(512, RW)):
                            w = min(512, RW)
                            nc.tensor.matmul(
                                ps[:, nb * w : (nb + 1) * w],
                                lhsT=wk_t[:, ec, :],
                                rhs=zt_t[ec][:, rnd * RW + nb * w : rnd * RW + (nb + 1) * w],
                                start=(ec == 0),
                                stop=(ec == EC - 1),
                            )
                    nc.vector.tensor_copy(kt[:, rnd * RW : (rnd + 1) * RW], ps[:, :RW])

                wq_t = attn.tile([P, EC, P], F32R, tag="wq")
                nc.sync.dma_start(wq_t[:], rearr(wq_d)[:, :, pair * P : (pair + 1) * P])
                qt = attn.tile([P, NQ], F32R, tag="qt")
                ps = big_ps.tile([P, BIGW], F32, tag="big")
                for ec in range(EC):
                    for nb in range(NQB):
                        nc.tensor.matmul(
                            ps[:, nb * QB : (nb + 1) * QB],
                            lhsT=wq_t[:, ec, :],
                            rhs=zt_t[ec][:, N - NQ + nb * QB : N - NQ + (nb + 1) * QB],
                            start=(ec == 0),
                            stop=(ec == EC - 1),
                        )
                nc.vector.tensor_copy(qt[:], ps[:, :NQ])
                return kt, qt

            def emit_attention(pair, kt, qt):
                vp = attn.tile([P, TC, 2, DA], F32R, tag="vp")
                nc.sync.dma_start(
                    vp[:],
                    vaug_dram[:, :, 2 * pair : 2 * pair + 2, :].rearrange(
                        "t p h a -> p t h a"
                    ),
                )
                for qb in range(NQB):
                    avs = []
                    for _hh in range(2):
                        av_t = av_ps.tile([DA, QB], F32, tag="av", name=f"av{_hh}")
                        avs.append(av_t)

                    def emit_av(kc, ex):
                        for hh in range(2):
                            nc.tensor.matmul(
                                avs[hh][:],
                                lhsT=vp[:, kc, hh, :],
                                rhs=ex[:, hh * QB : (hh + 1) * QB],
                                start=(kc == 0),
                                stop=(kc == TC - 1),
                            )

                    # AV for chunk kc-1 is emitted AFTER ST of chunk kc so the
                    # PE never head-of-line blocks on the exp of the current
                    # chunk: PE does ST(kc+1) while ACT runs exp(kc).
                    pend = None
                    for kc in range(TC):
                        st = big_ps.tile([P, BIGW], F32, tag="big")
                        for hh in range(2):
                            nc.tensor.matmul(
                                st[:, hh * 512 : hh * 512 + QB],
                                lhsT=kt[hh * DH : (hh + 1) * DH, kc * P : (kc + 1) * P],
                                rhs=qt[hh * DH : (hh + 1) * DH, qb * QB : (qb + 1) * QB],
                                start=True,
                                stop=True,
                                tile_position=(hh * DH, 0),
                            )
                        ex = expp.tile([P, 2 * QB], F32R, tag="ex")
                        if QB == 512:
                            nc.scalar.activation(
                                ex[:],
                                st[:, :1024],
                                mybir.ActivationFunctionType.Exp,
                                scale=float(SCALE),
                            )
                        else:
                            for hh in range(2):
                                nc.scalar.activation(
                                    ex[:, hh * QB : (hh + 1) * QB],
                                    st[:, hh * 512 : hh * 512 + QB],
                                    mybir.ActivationFunctionType.Exp,
                                    scale=float(SCALE),
                                )
                        if pend is not None:
                            emit_av(*pend)
                        pend = (kc, ex)
                    emit_av(*pend)
                    # denominators of both heads -> one PE broadcast + recip
                    d0 = tmp1.tile([1, 2 * QB], F32R, tag="d0")
                    for hh in range(2):
                        nc.vector.tensor_copy(
                            d0[:, hh * QB : (hh + 1) * QB], avs[hh][DH : DH + 1, :]
                        )
                    bc = big_ps.tile([P, BIGW], F32, tag="big")
                    for nb in range(2 * QB // min(512, 2 * QB)):
                        w = min(512, 2 * QB)
                        nc.tensor.matmul(
                            bc[0:DH, nb * w : (nb + 1) * w],
                            lhsT=ones_row[:],
                            rhs=d0[:, nb * w : (nb + 1) * w],
                            start=True,
                            stop=True,
                        )
                    dn = tmp1.tile([DH, 2 * QB], F32, tag="dn")
                    nc.vector.reciprocal(dn[:], bc[0:DH, 0 : 2 * QB])
                    for hh in range(2):
                        h = 2 * pair + hh
                        onr = tmp.tile([DH, QB], F32R, tag="onr")
                        nc.vector.tensor_mul(
                            onr[:], avs[hh][0:DH, :], dn[:, hh * QB : (hh + 1) * QB]
                        )
                        nc.sync.dma_start(
                            ot_dram[h * DH : (h + 1) * DH, qb * QB : (qb + 1) * QB],
                            onr[:],
                        )

            for _rep in range(REP):
              with (
                tc.tile_pool(name="big_ps", bufs=2, space="PSUM") as big_ps,
                tc.tile_pool(name="av_ps", bufs=4, space="PSUM") as av_ps,
              ):
                emit_v_half(0)
                for pair in range(PAIRS):
                    if pair == PAIRS // 2:
                        emit_v_half(1)
                    kt, qt = emit_kq(pair)
                    emit_attention(pair, kt, qt)

              # output projection: yt[m*128+j, q] = sum_e wo.T[e, m*128+j]*ot[e, q] + bo
              with tc.tile_pool(name="op_ps", bufs=EC, space="PSUM") as op_ps:
                OPW = min(512, NQ)
                wo_sb = const.tile([P, EC, EMB], F32R, tag="wo_sb", name="wo_sb")
                nc.sync.dma_start(wo_sb[:], rearr(wo_d))
                for qhb in range(NQ // OPW):
                    pss = []
                    for m in range(EC):
                        ps_t = op_ps.tile([P, OPW], F32, tag="op", name=f"op{m}")
                        pss.append(ps_t)
                    for ec in range(EC):
                        otc = stg.tile([P, OPW], F32R, tag="otc")
                        nc.sync.dma_start(
                            otc[:],
                            rearr(ot_dram[:])[:, ec, qhb * OPW : (qhb + 1) * OPW],
                        )
                        for m in range(EC):
                            nc.tensor.matmul(
                                pss[m][:],
                                lhsT=wo_sb[:, ec, m * P : (m + 1) * P],
                                rhs=otc[:],
                                start=(ec == 0),
                                stop=(ec == EC - 1),
                            )
                    for m in range(EC):
                        yt_t = tmp.tile([P, OPW], F32, tag="yt")
                        nc.vector.tensor_scalar(
                            yt_t[:], pss[m][:], bo_sb[:, m : m + 1], None,
                            op0=mybir.AluOpType.add,
                        )
                        nc.sync.dma_start(
                            yt_d[m * P : (m + 1) * P, qhb * OPW : (qhb + 1) * OPW],
                            yt_t[:],
                        )

    return nc


_COMPILED = {}


def get_compiled(cfg_name="full"):
    if cfg_name not in _COMPILED:
        cfg = full_cfg() if cfg_name == "full" else small_cfg()
        nc = bacc.Bacc("TRN2", target_bir_lowering=False, debug=False, num_devices=1)
        build_program(nc, cfg)
        nc.compile()
        _COMPILED[cfg_name] = nc
    return _COMPILED[cfg_name]


def make_in_maps(z, w_q, w_k, w_v, w_o, b_o):
    """Host-side shard: 8 cores = (batch, query-half). Query tokens are
    permuted to the end of the token axis so the query slice is SPMD-fixed."""
    B, N, EMB = z.shape
    NQ = N // 2
    wqT = np.ascontiguousarray(w_q.T.astype(np.float32))
    wkT = np.ascontiguousarray(w_k.T.astype(np.float32))
    wvT = np.ascontiguousarray(w_v.T.astype(np.float32))
    woT = np.ascontiguousarray(w_o.T.astype(np.float32))
    bo = np.ascontiguousarray(b_o.astype(np.float32))
    in_maps = []
    for c in range(8):
        b, qh = c // 2, c % 2
        zT = z[b].T.astype(np.float32)  # [EMB, N]
        if qh == 0:
            zp = np.concatenate([zT[:, NQ:], zT[:, :NQ]], axis=1)
        else:
            zp = zT
        in_maps.append(
            {"zt": np.ascontiguousarray(zp), "wq": wqT, "wk": wkT, "wv": wvT,
             "wo": woT, "bo": bo}
        )
    return in_maps


def kernel(z, w_q, w_k, w_v, w_o, b_o):
    B, N, EMB = z.shape
    NQ = N // 2
    nc = get_compiled("full")
    in_maps = make_in_maps(z, w_q, w_k, w_v, w_o, b_o)
    res = bass_utils.run_bass_kernel_spmd(nc, in_maps, core_ids=list(range(8)))
    y = np.empty((B, N, EMB), dtype=np.float32)
    for c in range(8):
        b, qh = c // 2, c % 2
        y[b, qh * NQ : (qh + 1) * NQ, :] = res.results[c]["yt"].T
    return y



# revision 2
# speedup vs baseline: 2.1613x; 2.1613x over previous
"""Multi-head self-attention TRN2 Bass kernel (8-core SPMD), v2.

Problem: z [4, 2048, 1024], w_q/w_k/w_v/w_o [1024, 1024] (torch Linear
convention: q = z @ w_q.T), b_o [1024]. 16 heads x 64 dims, softmax scale
1/sqrt(64).

Sharding: 8 cores = (4 batches) x (2 head-halves). Each core computes Q/K/V
projections, attention and the partial output projection for its 8 heads over
all 2048 tokens. w_q/w_k/w_v are column-sliced, w_o row-sliced; the host sums
the two partial y's per batch and adds b_o. No duplicated projection work and
no collectives.

Device-side layout: contraction dims live on partitions, all matmul operands
bf16 (fp32 PSUM accumulation). V (ones-augmented per head) and O^T stay
SBUF-resident. Softmax is unnormalized flash-style: exp(S) streams into the
AV matmul whose stationary V carries an all-ones column accumulating the
denominators; normalization happens at PSUM eviction (reciprocal on DVE,
partition-broadcast on the otherwise-idle GPSIMD engine).

Scheduling: the ACT engine (33.5M exps/core) is the steady-state floor, so
projection matmuls for the NEXT head pair are emitted as filler inside the
current pair's attention kc-loop — the PE works ahead while ACT chews the exp
stream, and ACT never drains at pair boundaries.
"""

import os
import sys

import numpy as np

for _p in ("/opt/trn_rl_repo", "/root/.axon_site/_ro/trn_rl_repo"):
    if os.path.isdir(_p) and _p not in sys.path:
        sys.path.insert(0, _p)

import ml_dtypes

import concourse.bacc as bacc
import concourse.mybir as mybir
import concourse.tile as tile
from concourse import bass_utils

F32 = mybir.dt.float32
F32R = mybir.dt.float32r
BF16 = mybir.dt.bfloat16
P = 128


def full_cfg():
    return dict(EMB=1024, N=2048, H=16, DH=64)


def build_program(nc, cfg):
    EMB, N, H, DH = cfg["EMB"], cfg["N"], cfg["H"], cfg["DH"]
    HPC = H // 2             # heads per core
    HD = HPC * DH            # head dims per core (512)
    EC = EMB // P            # emb contraction chunks (8)
    ECO = HD // P            # o-proj contraction chunks (4)
    MO = EMB // P            # o-proj output chunks (8)
    TC = N // P              # key-token chunks (16)
    PAIRS = HPC // 2         # head pairs per core (4)
    QB = 512                 # query block
    NQB = N // QB            # 4
    SCALE = 1.0 / np.sqrt(DH)
    DA = DH + 1              # V head dims + ones column

    zt_d = nc.dram_tensor("zt", [EMB, N], BF16, kind="ExternalInput").ap()
    wq_d = nc.dram_tensor("wq", [EMB, HD], BF16, kind="ExternalInput").ap()
    wk_d = nc.dram_tensor("wk", [EMB, HD], BF16, kind="ExternalInput").ap()
    wv_d = nc.dram_tensor("wv", [EMB, HD], BF16, kind="ExternalInput").ap()
    wo_d = nc.dram_tensor("wo", [HD, EMB], BF16, kind="ExternalInput").ap()
    yt_d = nc.dram_tensor("yt", [EMB, N], F32, kind="ExternalOutput").ap()

    def rearr(ap):  # [E, X] dram -> [P, E//P, X] partition view
        return ap.rearrange("(eo p) x -> p eo x", p=P)

    with tile.TileContext(nc) as tc:
        with (
            tc.tile_pool(name="const", bufs=1) as const,
            tc.tile_pool(name="vau", bufs=1) as vau,
            tc.tile_pool(name="attnw", bufs=2) as attnw,
            tc.tile_pool(name="kqp", bufs=2) as kqp,
            tc.tile_pool(name="expp", bufs=4) as expp,
            tc.tile_pool(name="otp", bufs=1) as otp,
            tc.tile_pool(name="tmp1", bufs=3) as tmp1,
        ):
            # DMA order tuned for fastest start: pair-0 K/Q weights, then the
            # first token-column quarter of z (enough for K/Q quarter 0), then
            # wv (needed by the V fillers inside qb0), then the rest.
            wk0 = attnw.tile([P, EC, P], BF16, tag="wk", name="wk0")
            nc.sync.dma_start(wk0[:], rearr(wk_d)[:, :, 0:P])
            wq0 = attnw.tile([P, EC, P], BF16, tag="wq", name="wq0")
            nc.sync.dma_start(wq0[:], rearr(wq_d)[:, :, 0:P])
            zt_t = [
                const.tile([P, N], BF16, tag=f"zt{_ec}", name=f"zt{_ec}")
                for _ec in range(EC)
            ]
            for _ec in range(EC):
                nc.sync.dma_start(zt_t[_ec][:, 0:512], rearr(zt_d)[:, _ec, 0:512])
            wv_sb = const.tile([P, EC, HD], BF16, tag="wv_sb", name="wv_sb")
            nc.sync.dma_start(wv_sb[:], rearr(wv_d))
            for _q in range(1, 4):
                for _ec in range(EC):
                    nc.sync.dma_start(
                        zt_t[_ec][:, _q * 512 : (_q + 1) * 512],
                        rearr(zt_d)[:, _ec, _q * 512 : (_q + 1) * 512],
                    )
            wo_sb = const.tile([P, ECO, EMB], BF16, tag="wo_sb", name="wo_sb")
            nc.sync.dma_start(wo_sb[:], rearr(wo_d))

            # vaug[:, tc, h8, :]: V tokens on partitions, per-head dims + ones
            vaug = vau.tile([P, TC, HPC, DA], BF16, tag="vaug", name="vaug")
            nc.any.memset(vaug[:, :, :, DH:DA], 1.0)

            # ot[:, pair, q]: O^T for this core's 512 dims, SBUF-resident
            ot = otp.tile([P, ECO, N], BF16, tag="ot", name="ot")

            def make_v_closure(tci, grp):
                # half the heads (256 cols) per closure: group 0 feeds pairs
                # 0-1 (needed in qb0), group 1 feeds pairs 2-3 (later)
                def f():
                    ps = kqv_ps.tile([P, HD // 2], F32, tag="kqv", name="vps")
                    for ec in range(EC):
                        nc.tensor.matmul(
                            ps[:],
                            lhsT=zt_t[ec][:, tci * P : (tci + 1) * P],
                            rhs=wv_sb[:, ec, grp * (HD // 2) : (grp + 1) * (HD // 2)],
                            start=(ec == 0),
                            stop=(ec == EC - 1),
                        )
                    nc.vector.tensor_copy(
                        vaug[:, tci, 4 * grp : 4 * (grp + 1), 0:DH],
                        ps[:].rearrange("p (h d) -> p h d", d=DH),
                    )
                return f

            def kq_setup(pair, wk_t=None, wq_t=None):
                """DMA the pair's K/Q weight slices; return (kt, qt, closures)
                with closures = [K q0..q3, Q q0..q3], each one quarter (512
                token columns) of the projection."""
                if wk_t is None:
                    wk_t = attnw.tile([P, EC, P], BF16, tag="wk", name="wk_t")
                    nc.sync.dma_start(
                        wk_t[:], rearr(wk_d)[:, :, pair * P : (pair + 1) * P]
                    )
                    wq_t = attnw.tile([P, EC, P], BF16, tag="wq", name="wq_t")
                    nc.sync.dma_start(
                        wq_t[:], rearr(wq_d)[:, :, pair * P : (pair + 1) * P]
                    )
                kt = kqp.tile([P, N], BF16, tag="kt", name="kt")
                qt = kqp.tile([P, N], BF16, tag="qt", name="qt")
                closures = []
                for w_t, dst in ((wk_t, kt), (wq_t, qt)):
                    for quarter in range(4):
                        def f(w_t=w_t, dst=dst, q=quarter):
                            ps = kqv_ps.tile([P, 512], F32, tag="kqv", name="kqps")
                            for ec in range(EC):
                                nc.tensor.matmul(
                                    ps[:],
                                    lhsT=w_t[:, ec, :],
                                    rhs=zt_t[ec][:, q * 512 : (q + 1) * 512],
                                    start=(ec == 0),
                                    stop=(ec == EC - 1),
                                )
                            nc.vector.tensor_copy(dst[:, q * 512 : (q + 1) * 512], ps[:])
                        closures.append(f)
                return kt, qt, closures

            def emit_attention(pair, kt, qt, sched, boundary):
                """sched[qb][slot] / boundary[qb]: filler closure lists."""
                for qb in range(NQB):
                    avs = []
                    for _hh in range(2):
                        av_t = av_ps.tile([DA, QB], F32, tag="av", name=f"av{_hh}")
                        avs.append(av_t)

                    def emit_av(kc, ex):
                        for hh in range(2):
                            nc.tensor.matmul(
                                avs[hh][:],
                                lhsT=vaug[:, kc, 2 * pair + hh, :],
                                rhs=ex[:, hh * QB : (hh + 1) * QB],
                                start=(kc == 0),
                                stop=(kc == TC - 1),
                            )

                    # AV for chunk kc-1 is emitted AFTER ST of chunk kc so the
                    # PE never head-of-line blocks on the exp of the current
                    # chunk: PE does ST(kc+1) while ACT runs exp(kc).
                    pend = None
                    for kc in range(TC):
                        st = big_ps.tile([P, 1024], F32, tag="big", name="st")
                        for hh in range(2):
                            nc.tensor.matmul(
                                st[:, hh * 512 : hh * 512 + QB],
                                lhsT=kt[hh * DH : (hh + 1) * DH, kc * P : (kc + 1) * P],
                                rhs=qt[hh * DH : (hh + 1) * DH, qb * QB : (qb + 1) * QB],
                                start=True,
                                stop=True,
                                tile_position=(hh * DH, 0),
                            )
                        ex = expp.tile([P, 2 * QB], BF16, tag="ex", name="ex")
                        nc.scalar.activation(
                            ex[:],
                            st[:, :1024],
                            mybir.ActivationFunctionType.Exp,
                            scale=float(SCALE),
                        )
                        for f in sched[qb].get(kc, ()):
                            f()
                        if pend is not None:
                            emit_av(*pend)
                        pend = (kc, ex)
                    emit_av(*pend)
                    for f in boundary[qb]:
                        f()
                    # denominators: reciprocal on DVE (straight from the AV
                    # PSUM ones-row), broadcast on GPSIMD
                    dr = tmp1.tile([1, 2 * QB], F32, tag="dr", name="dr")
                    for hh in range(2):
                        nc.vector.reciprocal(
                            dr[:, hh * QB : (hh + 1) * QB], avs[hh][DH : DH + 1, :]
                        )
                    dn = tmp1.tile([DH, 2 * QB], F32, tag="dn", name="dn")
                    nc.gpsimd.partition_broadcast(dn[:], dr[:])
                    for hh in range(2):
                        nc.vector.tensor_mul(
                            ot[hh * DH : (hh + 1) * DH, pair, qb * QB : (qb + 1) * QB],
                            avs[hh][0:DH, :],
                            dn[:, hh * QB : (hh + 1) * QB],
                        )

            def make_o_closure(qhb, m):
                def f():
                    ps = kqv_ps.tile([P, QB], F32, tag="kqv", name="ops")
                    for ec in range(ECO):
                        nc.tensor.matmul(
                            ps[:],
                            lhsT=wo_sb[:, ec, m * P : (m + 1) * P],
                            rhs=ot[:, ec, qhb * QB : (qhb + 1) * QB],
                            start=(ec == 0),
                            stop=(ec == ECO - 1),
                        )
                    yt_t = ytstg.tile([P, QB], F32, tag="yt", name="yt_t")
                    nc.vector.tensor_copy(yt_t[:], ps[:])
                    nc.sync.dma_start(
                        yt_d[m * P : (m + 1) * P, qhb * QB : (qhb + 1) * QB],
                        yt_t[:],
                    )
                return f

            with (
                tc.tile_pool(name="big_ps", bufs=2, space="PSUM") as big_ps,
                tc.tile_pool(name="av_ps", bufs=3, space="PSUM") as av_ps,
                tc.tile_pool(name="kqv_ps", bufs=1, space="PSUM") as kqv_ps,
                tc.tile_pool(name="ytstg", bufs=4) as ytstg,
            ):
                kt0, qt0, cl0 = kq_setup(0, wk0, wq0)
                cl0[0]()  # K quarter 0
                cl0[4]()  # Q quarter 0
                ktqt = (kt0, qt0)
                pend_setup = None  # closures of next pair scheduled into current
                for pair in range(PAIRS):
                    sched = {qb: {} for qb in range(NQB)}
                    boundary = {qb: [] for qb in range(NQB)}
                    kt, qt = ktqt
                    if pair == 0:
                        for tci in range(TC):
                            sched[0].setdefault(tci, []).append(make_v_closure(tci, 0))
                        # V group 1 (heads 4-7, first consumed by pair 2)
                        # spreads over qb1/qb2
                        for tci in range(TC):
                            qb_ = 1 + tci // 8
                            sched[qb_].setdefault(2 * (tci % 8), []).append(
                                make_v_closure(tci, 1)
                            )
                        # K quarters 1-3 of pair 0 inside qb0 (needed by kc
                        # 4/8/12); Q quarters at the qb boundaries.
                        sched[0].setdefault(1, []).append(cl0[1])
                        sched[0].setdefault(5, []).append(cl0[2])
                        sched[0].setdefault(9, []).append(cl0[3])
                        own = cl0
                    else:
                        own = pend_setup
                    boundary[0].append(own[5])
                    boundary[1].append(own[6])
                    boundary[2].append(own[7])
                    if pair < PAIRS - 1:
                        ktn, qtn, cln = kq_setup(pair + 1)
                        ktqt = (ktn, qtn)
                        pend_setup = cln
                        # carry-in for next pair: K q0-3 + Q q0 spread over
                        # this pair's qb1-3
                        sched[1].setdefault(3, []).append(cln[0])
                        sched[1].setdefault(11, []).append(cln[1])
                        sched[2].setdefault(3, []).append(cln[2])
                        sched[2].setdefault(11, []).append(cln[3])
                        sched[3].setdefault(3, []).append(cln[4])
                    else:
                        # last pair: pipeline the output projection's first
                        # three query blocks into qb1-3 (qhb's ot is complete
                        # once this pair's previous qb is normalized)
                        for qhb in range(3):
                            for m in range(MO):
                                sched[qhb + 1].setdefault(2 * m + 1, []).append(
                                    make_o_closure(qhb, m)
                                )
                    emit_attention(pair, kt, qt, sched, boundary)

            # output projection tail: last query block (first three were
            # pipelined into pair 3's attention)
            with (
                tc.tile_pool(name="op_ps", bufs=4, space="PSUM") as op_ps,
                tc.tile_pool(name="ytstg2", bufs=4) as ytstg2,
            ):
                qhb = N // QB - 1
                for m in range(MO):
                    ps_t = op_ps.tile([P, QB], F32, tag="op", name=f"op{m}")
                    for ec in range(ECO):
                        nc.tensor.matmul(
                            ps_t[:],
                            lhsT=wo_sb[:, ec, m * P : (m + 1) * P],
                            rhs=ot[:, ec, qhb * QB : (qhb + 1) * QB],
                            start=(ec == 0),
                            stop=(ec == ECO - 1),
                        )
                    yt_t = ytstg2.tile([P, QB], F32, tag="yt", name="yt_t")
                    nc.vector.tensor_copy(yt_t[:], ps_t[:])
                    nc.sync.dma_start(
                        yt_d[m * P : (m + 1) * P, qhb * QB : (qhb + 1) * QB],
                        yt_t[:],
                    )

    return nc


_COMPILED = {}


def get_compiled(cfg_name="full"):
    if cfg_name not in _COMPILED:
        cfg = full_cfg()
        nc = bacc.Bacc("TRN2", target_bir_lowering=False, debug=False, num_devices=1)
        build_program(nc, cfg)
        nc.compile()
        _COMPILED[cfg_name] = nc
    return _COMPILED[cfg_name]


def make_in_maps(z, w_q, w_k, w_v, w_o, b_o):
    """Host-side shard: 8 cores = (batch, head-half). Column-slice w_q/w_k/w_v
    and row-slice w_o per head half; all operands bf16."""
    B, N, EMB = z.shape
    HD = EMB // 2
    BF = ml_dtypes.bfloat16
    wqT = w_q.T.astype(BF)
    wkT = w_k.T.astype(BF)
    wvT = w_v.T.astype(BF)
    woT = w_o.T.astype(BF)
    in_maps = []
    for c in range(8):
        b, hh = c // 2, c % 2
        zT = np.ascontiguousarray(z[b].T.astype(BF))  # [EMB, N]
        in_maps.append(
            {
                "zt": zT,
                "wq": np.ascontiguousarray(wqT[:, hh * HD : (hh + 1) * HD]),
                "wk": np.ascontiguousarray(wkT[:, hh * HD : (hh + 1) * HD]),
                "wv": np.ascontiguousarray(wvT[:, hh * HD : (hh + 1) * HD]),
                "wo": np.ascontiguousarray(woT[hh * HD : (hh + 1) * HD, :]),
            }
        )
    return in_maps


def combine_outputs(results, z, w_q, w_k, w_v, w_o, b_o):
    """results: list of 8 per-core dicts with 'yt' [EMB, N]. Partial y's of the
    two head-halves sum; add b_o."""
    B, N, EMB = z.shape
    y = np.empty((B, N, EMB), dtype=np.float32)
    bo = b_o.astype(np.float32)
    for b in range(B):
        acc = np.asarray(results[2 * b]["yt"], dtype=np.float32) + np.asarray(
            results[2 * b + 1]["yt"], dtype=np.float32
        )
        y[b] = acc.T + bo
    return y


def kernel(z, w_q, w_k, w_v, w_o, b_o):
    nc = get_compiled("full")
    in_maps = make_in_maps(z, w_q, w_k, w_v, w_o, b_o)
    res = bass_utils.run_bass_kernel_spmd(nc, in_maps, core_ids=list(range(8)))
    return combine_outputs(res.results, z, w_q, w_k, w_v, w_o, b_o)
